# revision 34
# baseline (speedup 1.0000x reference)
"""Trainium2 Bass kernel for MemorySpatialAttention.

Math (per batch b):
  f = LeakyReLU_0.1(BN(conv(x)))  with conv = full-length dot -> x[N,L] @ W[L,H]
  sim = f_in @ f_mem^T  banded to |i-j| <= 8 (17 neighbors, clamped at edges)
  attn = softmax_band(sim);  out = 0.5*x + 0.5*(attn @ mem)

Sharding: data-parallel over batch B=8 -> one batch per NeuronCore, no
collectives.

v2 structure (one core, one batch):
- BN scale folded into W on host; conv bias shipped as a 57th contraction
  row (x/mem get an appended ones row), so the feature matmul emits the
  pre-activation directly and LeakyReLU is a single STT max(x, 0.1*x)
  spread across Vector/GpSimd (Scalar keeps only exp + psT copy).
- Per query-tile softmax: fused mask-add + row-max via tensor_tensor_reduce
  on Vector (per tile), batched subtract on GpSimd, batched exp on Scalar.
- Normalization: denominator rides the attn@mem matmul as an appended
  ones-column of memNB; epilogue is batched reciprocal + TT mult (bcast) +
  TT add (0.5x), replacing per-tile STTs.
- All 6 input DMAs issue in parallel on 6 different engine queues as the
  first instructions; output DMAs all ride the otherwise-idle Sync queue,
  per group, so transfers overlap remaining compute.
"""
import sys
sys.path.insert(0, '/opt/trn_rl_repo')

import numpy as np

B, N, C, L, H = 8, 2048, 1, 56, 128
LE = L + 1            # contraction rows incl. bias row
LEP = 64              # padded to 64 so input DMAs split across 16 engines
NB, HALF = 17, 8
RT = 112              # query rows per tile
WIN = 128             # key window per tile
T = (N + RT - 1) // RT  # 19 tiles (last partial: 32 rows)
GRP = 4
GROUPS = [(0, 7), (7, 6), (13, 6)]  # (t0, K)
NG = len(GROUPS)
NPAD = RT * T         # 2128
RATE = 0.5
BN_EPS = 1e-5
NEG_SLOPE = 0.1
USE_TTR = False
USE_VCOPY = False
FI_PAD = NPAD         # fiT cols (2128)
FM_PAD = HALF + N + (RT * (T - 1) + WIN - N)  # 8 + 2048 + 88 = 2144

_cache = {}


def _build_program():
    import concourse.bass as bass
    import concourse.bacc as bacc
    import concourse.tile as tile
    from concourse import mybir

    F32 = mybir.dt.float32
    F16 = mybir.dt.float16
    BF16 = mybir.dt.bfloat16
    AF = mybir.ActivationFunctionType
    AX = mybir.AxisListType
    from concourse.alu_op_type import AluOpType as ALU

    def bcast(ap_slice, n):
        return bass.AP(tensor=ap_slice.tensor, offset=ap_slice.offset,
                       ap=[*ap_slice.ap, [0, n]])

    def bcast_mid(ap_slice, k):
        # [112, 1, 128] slice -> [112, k, 128] with stride-0 middle dim
        ap = [list(d) for d in ap_slice.ap]
        assert ap[1][1] == 1, ap
        ap[1] = [0, k]
        return bass.AP(tensor=ap_slice.tensor, offset=ap_slice.offset, ap=ap)

    nc = bacc.Bacc("TRN2", target_bir_lowering=False, debug=False)

    xmA0a = nc.dram_tensor("xmA0a", [LEP, H + 512], F16, kind="ExternalInput")
    xmA0b = nc.dram_tensor("xmA0b", [LEP, 512], F16, kind="ExternalInput")
    xmA1 = nc.dram_tensor("xmA1", [LEP, 1024], F16, kind="ExternalInput")
    xmB = nc.dram_tensor("xmB", [LEP, N], F16, kind="ExternalInput")
    mi = nc.dram_tensor("mi", [RT, 3 * WIN + RT], BF16, kind="ExternalInput")
    memNB = nc.dram_tensor("memNB", [128, T * (L + 1)], BF16, kind="ExternalInput")
    xhp = nc.dram_tensor("xhp", [RT, T * L], F16, kind="ExternalInput")
    out = nc.dram_tensor("out", [RT, T * L], F16, kind="ExternalOutput")

    with tile.TileContext(nc) as tc:
        with tc.tile_pool(name="consts", bufs=1) as consts, \
             tc.tile_pool(name="work", bufs=4) as work, \
             tc.tile_pool(name="pbig", bufs=3, space="PSUM") as pbig, \
             tc.tile_pool(name="ptc", bufs=2, space="PSUM") as ptc:

            xmA0a_s = consts.tile([LEP, H + 512], F16)
            xmA0b_s = consts.tile([LEP, 512], F16)
            xmA1_s = consts.tile([LEP, 1024], F16)
            xmB_s = consts.tile([LEP, N], F16)
            wT_s = xmA0a_s[:, 0:H]
            mi_s = consts.tile([RT, 3, WIN], BF16)
            ident = consts.tile([RT, RT], BF16)
            memNB_s = consts.tile([128, T, L + 1], BF16)
            xh_s = consts.tile([RT, T, L], F16)
            fiT = consts.tile([H, FI_PAD], F16)
            fmT = consts.tile([H, FM_PAD], F16)
            simS = consts.tile([RT, T, WIN], F32)
            simB = consts.tile([RT, T, WIN], BF16)
            EB = consts.tile([RT, T, WIN], BF16)
            rowmax = consts.tile([RT, T], F32)
            rinv = consts.tile([RT, T], F32)
            tmpn = consts.tile([RT, T, L], F16)
            outn = consts.tile([RT, T, L], F16)

            # one input DMA per engine queue, issued up front in parallel;
            # xmA0 (weights + first x/mem chunk) on sync so the first
            # feature matmul unblocks earliest
            # Strict transfer priority: the shared DMA engines serve all
            # active queues, so later-needed bulk must queue BEHIND early
            # inputs rather than run in parallel with them.
            # sync: xmA0 (feeds chunk0) then xmB (chunks 2-3, needed last).
            # gpsimd: xmA1 (chunk 1) then mask/ident/memNB.
            # scalar: xh (epilogue) only, plus the act table load.
            nc.sync.dma_start(out=xmA0a_s, in_=xmA0a.ap())
            nc.scalar.dma_start(out=xmA0b_s, in_=xmA0b.ap())
            nc.gpsimd.dma_start(out=xmA1_s, in_=xmA1.ap())
            nc.gpsimd.dma_start(out=mi_s, in_=mi.ap()[:, 0:3 * WIN].rearrange(
                "p (t w) -> p t w", w=WIN))
            nc.gpsimd.dma_start(out=ident, in_=mi.ap()[:, 3 * WIN:3 * WIN + RT])
            nc.sync.dma_start(out=xmB_s, in_=xmB.ap())
            nc.gpsimd.dma_start(out=memNB_s, in_=memNB.ap().rearrange(
                "p (t d) -> p t d", d=L + 1))
            nc.gpsimd.dma_start(out=xh_s, in_=xhp.ap().rearrange(
                "p (t d) -> p t d", d=L))
            nc.vector.memset(fmT[:, 0:HALF], 0.0)
            nc.vector.memset(fmT[:, HALF + N:FM_PAD], 0.0)
            nc.vector.memset(fiT[:, N:FI_PAD], 0.0)

            # ---- features in 512-col chunks ----
            # psF = W'^T @ x_ext (bias via ones row); LeakyReLU via STT
            CH = 512

            MINI = 128

            def feat_part(dst, off, src, c0, c1):
                psF = pbig.tile([128, c1 - c0], F32, tag="pbig", name="psF")
                nc.tensor.matmul(psF, lhsT=wT_s, rhs=src, start=True, stop=True)
                nc.scalar.activation(dst[:, off + c0:off + c1], psF,
                                     AF.Prelu, alpha=NEG_SLOPE)

            def feat_chunk(q):  # q = 1..3 -> cols [512q, 512q+512) of x/mem
                for dst, off, m in ((fiT, 0, 0), (fmT, HALF, 1)):
                    if q == 1:
                        src = xmA1_s[:, CH * m:CH * (m + 1)]
                    else:
                        # xmB packed as [x c2 | mem c2 | x c3 | mem c3]
                        src = xmB_s[:, 1024 * (q - 2) + CH * m:
                                    1024 * (q - 2) + CH * (m + 1)]
                    psF = pbig.tile([128, CH], F32, tag="pbig", name="psF")
                    nc.tensor.matmul(psF, lhsT=wT_s, rhs=src, start=True, stop=True)
                    nc.scalar.activation(dst[:, off + CH * q:off + CH * (q + 1)],
                                         psF, AF.Prelu, alpha=NEG_SLOPE)

            # chunk 0 in two parts: a 128-col mini first so group 0's QK
            # unblocks early, then the 384-col remainder
            feat_part(fiT, 0, xmA0a_s[:, H:H + MINI], 0, MINI)
            feat_part(fmT, HALF, xmA0b_s[:, 0:MINI], 0, MINI)
            feat_part(fiT, 0, xmA0a_s[:, H + MINI:H + CH], MINI, CH)
            feat_part(fmT, HALF, xmA0b_s[:, MINI:CH], MINI, CH)
            feat_chunk(1)

            # ---- banded attention in groups ----
            for g, (t0, K) in enumerate(GROUPS):
                if g in (1, 2):
                    feat_chunk(g + 1)

                tiles = list(range(t0, t0 + K))

                psA = pbig.tile([RT, K, WIN], F32, tag="pbig", name="psA")
                for k, t in enumerate(tiles):
                    nc.tensor.matmul(psA[:, k, :], lhsT=fiT[:, RT * t:RT * (t + 1)],
                                     rhs=fmT[:, RT * t:RT * t + WIN],
                                     start=True, stop=True)

                # fused (psA + mask) + row-max, per tile
                if USE_TTR:
                    for k, t in enumerate(tiles):
                        mik = 0 if t == 0 else (2 if t == T - 1 else 1)
                        nc.vector.tensor_tensor_reduce(
                            out=simS[:, t, :], in0=psA[:, k, :],
                            in1=mi_s[:, mik, :], scale=1.0, scalar=-1e30,
                            op0=ALU.add, op1=ALU.max,
                            accum_out=rowmax[:, t:t + 1])
                else:
                    runs = []  # (k_start, count, mi)
                    for k, t in enumerate(tiles):
                        mik = 0 if t == 0 else (2 if t == T - 1 else 1)
                        if runs and runs[-1][2] == mik:
                            runs[-1][1] += 1
                        else:
                            runs.append([k, 1, mik])
                    for k0, cnt, mik in runs:
                        if cnt == 1:
                            nc.vector.tensor_tensor(
                                simS[:, t0 + k0:t0 + k0 + 1, :],
                                psA[:, k0:k0 + 1, :],
                                mi_s[:, mik:mik + 1, :], ALU.add)
                        else:
                            nc.vector.tensor_tensor(
                                simS[:, t0 + k0:t0 + k0 + cnt, :],
                                psA[:, k0:k0 + cnt, :],
                                bcast_mid(mi_s[:, mik:mik + 1, :], cnt), ALU.add)
                    nc.vector.reduce_max(
                        rowmax[:, t0:t0 + K], simS[:, t0:t0 + K, :],
                        axis=AX.X)
                # simB = simS - rowmax (batched, bf16 out)
                nc.gpsimd.tensor_tensor(
                    simB[:, t0:t0 + K, :], simS[:, t0:t0 + K, :],
                    bcast(rowmax[:, t0:t0 + K], WIN), ALU.subtract)
                nc.scalar.activation(EB[:, t0:t0 + K, :], simB[:, t0:t0 + K, :],
                                     AF.Exp)

                psT = ptc.tile([128, K, RT], BF16, tag="ptc")
                for k, t in enumerate(tiles):
                    nc.tensor.transpose(psT[:, k, :], EB[:, t, :], ident)
                attnT = work.tile([128, K, RT], BF16)
                if USE_VCOPY:
                    nc.vector.tensor_scalar_add(attnT[:, 0:K, :], psT[:, 0:K, :], 0.0)
                else:
                    nc.scalar.copy(attnT[:, 0:K, :], psT[:, 0:K, :])

                psC = ptc.tile([RT, K, L + 1], F32, tag="ptc")
                for k, t in enumerate(tiles):
                    nc.tensor.matmul(psC[:, k, :], lhsT=attnT[:, k, :],
                                     rhs=memNB_s[:, t, :], start=True, stop=True)

                # epilogue: out = psC * (1/denom) + 0.5*x, batched
                nc.vector.reciprocal(rinv[:, t0:t0 + K], psC[:, 0:K, L])
                nc.vector.tensor_tensor(
                    tmpn[:, t0:t0 + K, :], psC[:, 0:K, 0:L],
                    bcast(rinv[:, t0:t0 + K], L), ALU.mult)
                nc.gpsimd.tensor_tensor(
                    outn[:, t0:t0 + K, :], tmpn[:, t0:t0 + K, :],
                    xh_s[:, t0:t0 + K, :], ALU.add)

                nc.sync.dma_start(
                    out=out.ap().rearrange("p (t d) -> p t d", d=L)[:, t0:t0 + K, :],
                    in_=outn[:, t0:t0 + K, :])

    nc.compile()
    return nc


def _host_prep(input, state_memory, conv_w, conv_b, bn_gamma, bn_beta, bn_mean, bn_var):
    from ml_dtypes import bfloat16

    s = (bn_gamma / np.sqrt(bn_var + BN_EPS)).astype(np.float32)
    bias_h = ((conv_b - bn_mean) * s + bn_beta).astype(np.float32)
    # W' = W * s (per output channel), bias as 57th contraction row
    wT = (conv_w[:, 0, :].T * s[None, :]).astype(np.float32)           # [L, H]
    wT_ext = np.concatenate(
        [wT, bias_h[None, :], np.zeros((LEP - LE, H), np.float32)], axis=0)

    # Per-tile additive mask [RT, WIN] (0 in band, -1e10 outside)
    def tile_mask(t):
        m = np.full((RT, WIN), -1e10, dtype=np.float32)
        for r in range(RT):
            i = RT * t + r
            if i >= N:
                continue
            lo = max(i - HALF, 0) - (RT * t - HALF)
            hi = min(i + HALF, N - 1) - (RT * t - HALF)
            m[r, lo:hi + 1] = 0.0
        return m

    maskD = np.stack([tile_mask(0), tile_mask(1), tile_mask(T - 1)], axis=1)
    maskD = maskD.reshape(RT, -1)                                      # [RT, 3*WIN]
    mi_host = np.concatenate([maskD, np.eye(RT, dtype=np.float32)], axis=1)

    in_maps = []
    for b in range(B):
        x = np.ascontiguousarray(input[b, :, 0, :]).astype(np.float32)
        mem = np.ascontiguousarray(state_memory[b, :, 0, :]).astype(np.float32)
        ones = np.ones((1, N), dtype=np.float32)
        zpad = np.zeros((LEP - LE, N), dtype=np.float32)
        xT = np.concatenate([x.T, ones, zpad], axis=0)                 # [LEP, N]
        memT = np.concatenate([mem.T, ones, zpad], axis=0)             # [LEP, N]
        # window-aligned mem blocks: block t = rows [RT*t-8, RT*t+120)
        mnb = np.zeros((T, 128, L + 1), dtype=np.float32)
        half_mem = (1.0 - RATE) * mem
        for t in range(T):
            lo = RT * t - HALF
            a, bnd = max(0, lo), min(N, lo + 128)
            if a < bnd:
                mnb[t, a - lo:bnd - lo, 0:L] = half_mem[a:bnd]
                mnb[t, a - lo:bnd - lo, L] = 1.0
        xh = np.zeros((T, RT, L), dtype=np.float32)
        xh.reshape(-1, L)[:N] = RATE * x
        in_maps.append({
            "xmA0a": np.ascontiguousarray(np.concatenate(
                [wT_ext, xT[:, 0:512]], axis=1)).astype(np.float16),
            "xmA0b": np.ascontiguousarray(memT[:, 0:512]).astype(np.float16),
            "xmA1": np.ascontiguousarray(np.concatenate(
                [xT[:, 512:1024], memT[:, 512:1024]], axis=1)).astype(np.float16),
            "xmB": np.ascontiguousarray(np.concatenate(
                [xT[:, 1024:1536], memT[:, 1024:1536],
                 xT[:, 1536:2048], memT[:, 1536:2048]], axis=1)).astype(np.float16),
            "mi": np.ascontiguousarray(mi_host).astype(bfloat16),
            "memNB": np.ascontiguousarray(
                mnb.transpose(1, 0, 2).reshape(128, -1)).astype(bfloat16),
            "xhp": np.ascontiguousarray(
                xh.transpose(1, 0, 2).reshape(RT, -1)).astype(np.float16),
        })
    return in_maps


def run(inputs, trace=False):
    from concourse.bass_utils import run_bass_kernel_spmd
    if "nc" not in _cache:
        _cache["nc"] = _build_program()
    nc = _cache["nc"]
    in_maps = _host_prep(**inputs)
    res = run_bass_kernel_spmd(nc, in_maps, core_ids=list(range(B)), trace=trace)
    out = np.empty((B, N, C, L), dtype=np.float32)
    for b in range(B):
        o = res.results[b]["out"].astype(np.float32).reshape(RT, T, L).transpose(1, 0, 2)
        out[b] = o.reshape(NPAD, L)[:N].reshape(N, C, L)
    return out, res


def kernel(**inputs):
    out, _ = run(inputs, trace=False)
    return out


# revision 35
# speedup vs baseline: 1.1335x; 1.1335x over previous
"""Trainium2 Bass kernel for MemorySpatialAttention.

Math (per batch b):
  f = LeakyReLU_0.1(BN(conv(x)))  with conv = full-length dot -> x[N,L] @ W[L,H]
  sim = f_in @ f_mem^T  banded to |i-j| <= 8 (17 neighbors, clamped at edges)
  attn = softmax_band(sim);  out = 0.5*x + 0.5*(attn @ mem)

Sharding: data-parallel over batch B=8 -> one batch per NeuronCore, no
collectives.

v2 structure (one core, one batch):
- BN scale folded into W on host; conv bias shipped as a 57th contraction
  row (x/mem get an appended ones row), so the feature matmul emits the
  pre-activation directly and LeakyReLU is a single STT max(x, 0.1*x)
  spread across Vector/GpSimd (Scalar keeps only exp + psT copy).
- Per query-tile softmax: fused mask-add + row-max via tensor_tensor_reduce
  on Vector (per tile), batched subtract on GpSimd, batched exp on Scalar.
- Normalization: denominator rides the attn@mem matmul as an appended
  ones-column of memNB; epilogue is batched reciprocal + TT mult (bcast) +
  TT add (0.5x), replacing per-tile STTs.
- All 6 input DMAs issue in parallel on 6 different engine queues as the
  first instructions; output DMAs all ride the otherwise-idle Sync queue,
  per group, so transfers overlap remaining compute.
"""
import sys
sys.path.insert(0, '/opt/trn_rl_repo')

import numpy as np

B, N, C, L, H = 8, 2048, 1, 56, 128
LE = L + 1            # contraction rows incl. bias row
LEP = 64              # padded to 64 so input DMAs split across 16 engines
NB, HALF = 17, 8
RT = 112              # query rows per tile
WIN = 128             # key window per tile
T = (N + RT - 1) // RT  # 19 tiles (last partial: 32 rows)
GRP = 4
GROUPS = [(0, 4), (4, 4), (8, 4), (12, 4), (16, 3)]  # (t0, K)
NG = len(GROUPS)
NPAD = RT * T         # 2128
RATE = 0.5
BN_EPS = 1e-5
NEG_SLOPE = 0.1
USE_TTR = False
USE_VCOPY = False
FI_PAD = NPAD         # fiT cols (2128)
FM_PAD = HALF + N + (RT * (T - 1) + WIN - N)  # 8 + 2048 + 88 = 2144

_cache = {}


def _build_program():
    import concourse.bass as bass
    import concourse.bacc as bacc
    import concourse.tile as tile
    from concourse import mybir

    F32 = mybir.dt.float32
    F16 = mybir.dt.float16
    BF16 = mybir.dt.bfloat16
    AF = mybir.ActivationFunctionType
    AX = mybir.AxisListType
    from concourse.alu_op_type import AluOpType as ALU

    def bcast(ap_slice, n):
        return bass.AP(tensor=ap_slice.tensor, offset=ap_slice.offset,
                       ap=[*ap_slice.ap, [0, n]])

    def bcast_mid(ap_slice, k):
        # [112, 1, 128] slice -> [112, k, 128] with stride-0 middle dim
        ap = [list(d) for d in ap_slice.ap]
        assert ap[1][1] == 1, ap
        ap[1] = [0, k]
        return bass.AP(tensor=ap_slice.tensor, offset=ap_slice.offset, ap=ap)

    nc = bacc.Bacc("TRN2", target_bir_lowering=False, debug=False)

    xmA0a = nc.dram_tensor("xmA0a", [LEP, H + 512], F16, kind="ExternalInput")
    xmA0b = nc.dram_tensor("xmA0b", [LEP, 512], F16, kind="ExternalInput")
    xmA1 = nc.dram_tensor("xmA1", [LEP, 1024], F16, kind="ExternalInput")
    xmB = nc.dram_tensor("xmB", [LEP, N], F16, kind="ExternalInput")
    mi = nc.dram_tensor("mi", [RT, 3 * WIN + RT], BF16, kind="ExternalInput")
    memNB = nc.dram_tensor("memNB", [128, T * (L + 1)], BF16, kind="ExternalInput")
    xhp = nc.dram_tensor("xhp", [RT, T * L], F16, kind="ExternalInput")
    out = nc.dram_tensor("out", [RT, T * L], F16, kind="ExternalOutput")

    with tile.TileContext(nc) as tc:
        with tc.tile_pool(name="consts", bufs=1) as consts, \
             tc.tile_pool(name="work", bufs=4) as work, \
             tc.tile_pool(name="pbig", bufs=4, space="PSUM") as pbig, \
             tc.tile_pool(name="ptc", bufs=3, space="PSUM") as ptc:

            xmA0a_s = consts.tile([LEP, H + 512], F16)
            xmA0b_s = consts.tile([LEP, 512], F16)
            xmA1_s = consts.tile([LEP, 1024], F16)
            xmB_s = consts.tile([LEP, N], F16)
            wT_s = xmA0a_s[:, 0:H]
            mi_s = consts.tile([RT, 3, WIN], BF16)
            ident = consts.tile([RT, RT], BF16)
            memNB_s = consts.tile([128, T, L + 1], BF16)
            xh_s = consts.tile([RT, T, L], F16)
            fiT = consts.tile([H, FI_PAD], F16)
            fmT = consts.tile([H, FM_PAD], F16)
            simS = consts.tile([RT, T, WIN], F32)
            simB = consts.tile([RT, T, WIN], BF16)
            EB = consts.tile([RT, T, WIN], BF16)
            rowmax = consts.tile([RT, T], F32)
            rinv = consts.tile([RT, T], F32)
            tmpn = consts.tile([RT, T, L], F16)
            outn = consts.tile([RT, T, L], F16)

            # one input DMA per engine queue, issued up front in parallel;
            # xmA0 (weights + first x/mem chunk) on sync so the first
            # feature matmul unblocks earliest
            # Strict transfer priority: the shared DMA engines serve all
            # active queues, so later-needed bulk must queue BEHIND early
            # inputs rather than run in parallel with them.
            # sync: xmA0 (feeds chunk0) then xmB (chunks 2-3, needed last).
            # gpsimd: xmA1 (chunk 1) then mask/ident/memNB.
            # scalar: xh (epilogue) only, plus the act table load.
            nc.sync.dma_start(out=xmA0a_s, in_=xmA0a.ap())
            nc.scalar.dma_start(out=xmA0b_s, in_=xmA0b.ap())
            nc.gpsimd.dma_start(out=xmA1_s, in_=xmA1.ap())
            nc.gpsimd.dma_start(out=mi_s, in_=mi.ap()[:, 0:3 * WIN].rearrange(
                "p (t w) -> p t w", w=WIN))
            nc.gpsimd.dma_start(out=ident, in_=mi.ap()[:, 3 * WIN:3 * WIN + RT])
            nc.gpsimd.dma_start(out=memNB_s, in_=memNB.ap().rearrange(
                "p (t d) -> p t d", d=L + 1))
            nc.gpsimd.dma_start(out=xh_s, in_=xhp.ap().rearrange(
                "p (t d) -> p t d", d=L))
            nc.vector.memset(fmT[:, 0:HALF], 0.0)
            nc.vector.memset(fmT[:, HALF + N:FM_PAD], 0.0)
            nc.vector.memset(fiT[:, N:FI_PAD], 0.0)

            # ---- features in 512-col chunks ----
            # psF = W'^T @ x_ext (bias via ones row); LeakyReLU via STT
            CH = 512

            MINI = 128

            def feat_part(dst, off, src, c0, c1):
                psF = pbig.tile([128, c1 - c0], F32, tag="pbig", name="psF")
                nc.tensor.matmul(psF, lhsT=wT_s, rhs=src, start=True, stop=True)
                nc.scalar.activation(dst[:, off + c0:off + c1], psF,
                                     AF.Prelu, alpha=NEG_SLOPE)

            def feat_chunk(q):  # q = 1..3 -> cols [512q, 512q+512) of x/mem
                for dst, off, m in ((fiT, 0, 0), (fmT, HALF, 1)):
                    if q == 1:
                        src = xmA1_s[:, CH * m:CH * (m + 1)]
                    else:
                        # xmB packed as [x c2 | mem c2 | x c3 | mem c3]
                        src = xmB_s[:, 1024 * (q - 2) + CH * m:
                                    1024 * (q - 2) + CH * (m + 1)]
                    psF = pbig.tile([128, CH], F32, tag="pbig", name="psF")
                    nc.tensor.matmul(psF, lhsT=wT_s, rhs=src, start=True, stop=True)
                    nc.scalar.activation(dst[:, off + CH * q:off + CH * (q + 1)],
                                         psF, AF.Prelu, alpha=NEG_SLOPE)

            # chunk 0 in two parts: a 128-col mini first so group 0's QK
            # unblocks early, then the 384-col remainder
            feat_part(fiT, 0, xmA0a_s[:, H:H + MINI], 0, MINI)
            feat_part(fmT, HALF, xmA0b_s[:, 0:MINI], 0, MINI)
            feat_part(fiT, 0, xmA0a_s[:, H + MINI:H + CH], MINI, CH)
            feat_part(fmT, HALF, xmA0b_s[:, MINI:CH], MINI, CH)
            feat_chunk(1)
            nc.scalar.dma_start(out=xmB_s, in_=xmB.ap())

            # ---- banded attention in groups ----
            for g, (t0, K) in enumerate(GROUPS):
                if g in (1, 2):
                    feat_chunk(g + 1)

                tiles = list(range(t0, t0 + K))

                psA = pbig.tile([RT, K, WIN], F32, tag="pbig", name="psA")
                for k, t in enumerate(tiles):
                    nc.tensor.matmul(psA[:, k, :], lhsT=fiT[:, RT * t:RT * (t + 1)],
                                     rhs=fmT[:, RT * t:RT * t + WIN],
                                     start=True, stop=True)

                # fused (psA + mask) + row-max, per tile
                if USE_TTR:
                    for k, t in enumerate(tiles):
                        mik = 0 if t == 0 else (2 if t == T - 1 else 1)
                        nc.vector.tensor_tensor_reduce(
                            out=simS[:, t, :], in0=psA[:, k, :],
                            in1=mi_s[:, mik, :], scale=1.0, scalar=-1e30,
                            op0=ALU.add, op1=ALU.max,
                            accum_out=rowmax[:, t:t + 1])
                else:
                    runs = []  # (k_start, count, mi)
                    for k, t in enumerate(tiles):
                        mik = 0 if t == 0 else (2 if t == T - 1 else 1)
                        if runs and runs[-1][2] == mik:
                            runs[-1][1] += 1
                        else:
                            runs.append([k, 1, mik])
                    for k0, cnt, mik in runs:
                        if cnt == 1:
                            nc.vector.tensor_tensor(
                                simS[:, t0 + k0:t0 + k0 + 1, :],
                                psA[:, k0:k0 + 1, :],
                                mi_s[:, mik:mik + 1, :], ALU.add)
                        else:
                            nc.vector.tensor_tensor(
                                simS[:, t0 + k0:t0 + k0 + cnt, :],
                                psA[:, k0:k0 + cnt, :],
                                bcast_mid(mi_s[:, mik:mik + 1, :], cnt), ALU.add)
                    nc.vector.reduce_max(
                        rowmax[:, t0:t0 + K], simS[:, t0:t0 + K, :],
                        axis=AX.X)
                # simB = simS - rowmax (batched, bf16 out)
                nc.gpsimd.tensor_tensor(
                    simB[:, t0:t0 + K, :], simS[:, t0:t0 + K, :],
                    bcast(rowmax[:, t0:t0 + K], WIN), ALU.subtract)
                nc.scalar.activation(EB[:, t0:t0 + K, :], simB[:, t0:t0 + K, :],
                                     AF.Exp)

                psT = ptc.tile([128, K, RT], BF16, tag="ptc")
                for k, t in enumerate(tiles):
                    nc.tensor.transpose(psT[:, k, :], EB[:, t, :], ident)
                attnT = work.tile([128, K, RT], BF16)
                if USE_VCOPY:
                    nc.vector.tensor_scalar_add(attnT[:, 0:K, :], psT[:, 0:K, :], 0.0)
                else:
                    nc.scalar.copy(attnT[:, 0:K, :], psT[:, 0:K, :])

                psC = ptc.tile([RT, K, L + 1], F32, tag="ptc")
                for k, t in enumerate(tiles):
                    nc.tensor.matmul(psC[:, k, :], lhsT=attnT[:, k, :],
                                     rhs=memNB_s[:, t, :], start=True, stop=True)

                # epilogue: out = psC * (1/denom) + 0.5*x, batched
                nc.vector.reciprocal(rinv[:, t0:t0 + K], psC[:, 0:K, L])
                nc.vector.tensor_tensor(
                    tmpn[:, t0:t0 + K, :], psC[:, 0:K, 0:L],
                    bcast(rinv[:, t0:t0 + K], L), ALU.mult)
                nc.gpsimd.tensor_tensor(
                    outn[:, t0:t0 + K, :], tmpn[:, t0:t0 + K, :],
                    xh_s[:, t0:t0 + K, :], ALU.add)

                nc.sync.dma_start(
                    out=out.ap().rearrange("p (t d) -> p t d", d=L)[:, t0:t0 + K, :],
                    in_=outn[:, t0:t0 + K, :])

    nc.compile()
    return nc


def _host_prep(input, state_memory, conv_w, conv_b, bn_gamma, bn_beta, bn_mean, bn_var):
    from ml_dtypes import bfloat16

    s = (bn_gamma / np.sqrt(bn_var + BN_EPS)).astype(np.float32)
    bias_h = ((conv_b - bn_mean) * s + bn_beta).astype(np.float32)
    # W' = W * s (per output channel), bias as 57th contraction row
    wT = (conv_w[:, 0, :].T * s[None, :]).astype(np.float32)           # [L, H]
    wT_ext = np.concatenate(
        [wT, bias_h[None, :], np.zeros((LEP - LE, H), np.float32)], axis=0)

    # Per-tile additive mask [RT, WIN] (0 in band, -1e10 outside)
    def tile_mask(t):
        m = np.full((RT, WIN), -1e10, dtype=np.float32)
        for r in range(RT):
            i = RT * t + r
            if i >= N:
                continue
            lo = max(i - HALF, 0) - (RT * t - HALF)
            hi = min(i + HALF, N - 1) - (RT * t - HALF)
            m[r, lo:hi + 1] = 0.0
        return m

    maskD = np.stack([tile_mask(0), tile_mask(1), tile_mask(T - 1)], axis=1)
    maskD = maskD.reshape(RT, -1)                                      # [RT, 3*WIN]
    mi_host = np.concatenate([maskD, np.eye(RT, dtype=np.float32)], axis=1)

    in_maps = []
    for b in range(B):
        x = np.ascontiguousarray(input[b, :, 0, :]).astype(np.float32)
        mem = np.ascontiguousarray(state_memory[b, :, 0, :]).astype(np.float32)
        ones = np.ones((1, N), dtype=np.float32)
        zpad = np.zeros((LEP - LE, N), dtype=np.float32)
        xT = np.concatenate([x.T, ones, zpad], axis=0)                 # [LEP, N]
        memT = np.concatenate([mem.T, ones, zpad], axis=0)             # [LEP, N]
        # window-aligned mem blocks: block t = rows [RT*t-8, RT*t+120)
        mnb = np.zeros((T, 128, L + 1), dtype=np.float32)
        half_mem = (1.0 - RATE) * mem
        for t in range(T):
            lo = RT * t - HALF
            a, bnd = max(0, lo), min(N, lo + 128)
            if a < bnd:
                mnb[t, a - lo:bnd - lo, 0:L] = half_mem[a:bnd]
                mnb[t, a - lo:bnd - lo, L] = 1.0
        xh = np.zeros((T, RT, L), dtype=np.float32)
        xh.reshape(-1, L)[:N] = RATE * x
        in_maps.append({
            "xmA0a": np.ascontiguousarray(np.concatenate(
                [wT_ext, xT[:, 0:512]], axis=1)).astype(np.float16),
            "xmA0b": np.ascontiguousarray(memT[:, 0:512]).astype(np.float16),
            "xmA1": np.ascontiguousarray(np.concatenate(
                [xT[:, 512:1024], memT[:, 512:1024]], axis=1)).astype(np.float16),
            "xmB": np.ascontiguousarray(np.concatenate(
                [xT[:, 1024:1536], memT[:, 1024:1536],
                 xT[:, 1536:2048], memT[:, 1536:2048]], axis=1)).astype(np.float16),
            "mi": np.ascontiguousarray(mi_host).astype(bfloat16),
            "memNB": np.ascontiguousarray(
                mnb.transpose(1, 0, 2).reshape(128, -1)).astype(bfloat16),
            "xhp": np.ascontiguousarray(
                xh.transpose(1, 0, 2).reshape(RT, -1)).astype(np.float16),
        })
    return in_maps


def run(inputs, trace=False):
    from concourse.bass_utils import run_bass_kernel_spmd
    if "nc" not in _cache:
        _cache["nc"] = _build_program()
    nc = _cache["nc"]
    in_maps = _host_prep(**inputs)
    res = run_bass_kernel_spmd(nc, in_maps, core_ids=list(range(B)), trace=trace)
    out = np.empty((B, N, C, L), dtype=np.float32)
    for b in range(B):
        o = res.results[b]["out"].astype(np.float32).reshape(RT, T, L).transpose(1, 0, 2)
        out[b] = o.reshape(NPAD, L)[:N].reshape(N, C, L)
    return out, res


def kernel(**inputs):
    out, _ = run(inputs, trace=False)
    return out


# revision 36
# speedup vs baseline: 1.1535x; 1.0177x over previous
"""Trainium2 Bass kernel for MemorySpatialAttention.

Math (per batch b):
  f = LeakyReLU_0.1(BN(conv(x)))  with conv = full-length dot -> x[N,L] @ W[L,H]
  sim = f_in @ f_mem^T  banded to |i-j| <= 8 (17 neighbors, clamped at edges)
  attn = softmax_band(sim);  out = 0.5*x + 0.5*(attn @ mem)

Sharding: data-parallel over batch B=8 -> one batch per NeuronCore, no
collectives.

v2 structure (one core, one batch):
- BN scale folded into W on host; conv bias shipped as a 57th contraction
  row (x/mem get an appended ones row), so the feature matmul emits the
  pre-activation directly and LeakyReLU is a single STT max(x, 0.1*x)
  spread across Vector/GpSimd (Scalar keeps only exp + psT copy).
- Per query-tile softmax: fused mask-add + row-max via tensor_tensor_reduce
  on Vector (per tile), batched subtract on GpSimd, batched exp on Scalar.
- Normalization: denominator rides the attn@mem matmul as an appended
  ones-column of memNB; epilogue is batched reciprocal + TT mult (bcast) +
  TT add (0.5x), replacing per-tile STTs.
- All 6 input DMAs issue in parallel on 6 different engine queues as the
  first instructions; output DMAs all ride the otherwise-idle Sync queue,
  per group, so transfers overlap remaining compute.
"""
import sys
sys.path.insert(0, '/opt/trn_rl_repo')

import numpy as np

B, N, C, L, H = 8, 2048, 1, 56, 128
LE = L + 1            # contraction rows incl. bias row
LEP = 64              # padded to 64 so input DMAs split across 16 engines
NB, HALF = 17, 8
RT = 112              # query rows per tile
WIN = 128             # key window per tile
T = (N + RT - 1) // RT  # 19 tiles (last partial: 32 rows)
GRP = 4
GROUPS = [(0, 4), (4, 4), (8, 4), (12, 4), (16, 3)]  # (t0, K)
NG = len(GROUPS)
NPAD = RT * T         # 2128
RATE = 0.5
BN_EPS = 1e-5
NEG_SLOPE = 0.1
USE_TTR = False
USE_VCOPY = False
FI_PAD = NPAD         # fiT cols (2128)
FM_PAD = HALF + N + (RT * (T - 1) + WIN - N)  # 8 + 2048 + 88 = 2144

_cache = {}


def _build_program():
    import concourse.bass as bass
    import concourse.bacc as bacc
    import concourse.tile as tile
    from concourse import mybir

    F32 = mybir.dt.float32
    F16 = mybir.dt.float16
    BF16 = mybir.dt.bfloat16
    AF = mybir.ActivationFunctionType
    AX = mybir.AxisListType
    from concourse.alu_op_type import AluOpType as ALU

    def bcast(ap_slice, n):
        return bass.AP(tensor=ap_slice.tensor, offset=ap_slice.offset,
                       ap=[*ap_slice.ap, [0, n]])

    def bcast_mid(ap_slice, k):
        # [112, 1, 128] slice -> [112, k, 128] with stride-0 middle dim
        ap = [list(d) for d in ap_slice.ap]
        assert ap[1][1] == 1, ap
        ap[1] = [0, k]
        return bass.AP(tensor=ap_slice.tensor, offset=ap_slice.offset, ap=ap)

    nc = bacc.Bacc("TRN2", target_bir_lowering=False, debug=False)

    xmA0a = nc.dram_tensor("xmA0a", [LEP, H + 512], F16, kind="ExternalInput")
    xmA0b = nc.dram_tensor("xmA0b", [LEP, 512], F16, kind="ExternalInput")
    xmA1 = nc.dram_tensor("xmA1", [LEP, 1024], F16, kind="ExternalInput")
    xmB = nc.dram_tensor("xmB", [LEP, N], F16, kind="ExternalInput")
    mi = nc.dram_tensor("mi", [RT, 3 * WIN + RT], BF16, kind="ExternalInput")
    memNB = nc.dram_tensor("memNB", [128, T * (L + 1)], BF16, kind="ExternalInput")
    xhp = nc.dram_tensor("xhp", [RT, T * L], F16, kind="ExternalInput")
    out = nc.dram_tensor("out", [RT, T * L], F16, kind="ExternalOutput")

    with tile.TileContext(nc) as tc:
        with tc.tile_pool(name="consts", bufs=1) as consts, \
             tc.tile_pool(name="work", bufs=4) as work, \
             tc.tile_pool(name="pbig", bufs=4, space="PSUM") as pbig, \
             tc.tile_pool(name="ptc", bufs=3, space="PSUM") as ptc:

            xmA0a_s = consts.tile([LEP, H + 512], F16)
            xmA0b_s = consts.tile([LEP, 512], F16)
            xmA1_s = consts.tile([LEP, 1024], F16)
            xmB_s = consts.tile([LEP, N], F16)
            wT_s = xmA0a_s[:, 0:H]
            mi_s = consts.tile([RT, 3, WIN], BF16)
            ident = consts.tile([RT, RT], BF16)
            memNB_s = consts.tile([128, T, L + 1], BF16)
            xh_s = consts.tile([RT, T, L], F16)
            fiT = consts.tile([H, FI_PAD], F16)
            fmT = consts.tile([H, FM_PAD], F16)
            simS = consts.tile([RT, T, WIN], F32)
            simB = consts.tile([RT, T, WIN], BF16)
            EB = consts.tile([RT, T, WIN], BF16)
            rowmax = consts.tile([RT, T], F32)
            rinv = consts.tile([RT, T], F32)
            tmpn = consts.tile([RT, T, L], F16)
            outn = consts.tile([RT, T, L], F16)

            # one input DMA per engine queue, issued up front in parallel;
            # xmA0 (weights + first x/mem chunk) on sync so the first
            # feature matmul unblocks earliest
            # Strict transfer priority: the shared DMA engines serve all
            # active queues, so later-needed bulk must queue BEHIND early
            # inputs rather than run in parallel with them.
            # sync: xmA0 (feeds chunk0) then xmB (chunks 2-3, needed last).
            # gpsimd: xmA1 (chunk 1) then mask/ident/memNB.
            # scalar: xh (epilogue) only, plus the act table load.
            nc.sync.dma_start(out=xmA0a_s, in_=xmA0a.ap())
            nc.scalar.dma_start(out=xmA0b_s, in_=xmA0b.ap())
            nc.gpsimd.dma_start(out=xmA1_s, in_=xmA1.ap())
            nc.gpsimd.dma_start(out=mi_s, in_=mi.ap()[:, 0:3 * WIN].rearrange(
                "p (t w) -> p t w", w=WIN))
            nc.gpsimd.dma_start(out=ident, in_=mi.ap()[:, 3 * WIN:3 * WIN + RT])
            nc.gpsimd.dma_start(out=memNB_s, in_=memNB.ap().rearrange(
                "p (t d) -> p t d", d=L + 1))
            nc.gpsimd.dma_start(out=xh_s, in_=xhp.ap().rearrange(
                "p (t d) -> p t d", d=L))
            nc.vector.memset(fmT[:, 0:HALF], 0.0)
            nc.vector.memset(fmT[:, HALF + N:FM_PAD], 0.0)
            nc.vector.memset(fiT[:, N:FI_PAD], 0.0)

            # ---- features in 512-col chunks ----
            # psF = W'^T @ x_ext (bias via ones row); LeakyReLU via STT
            CH = 512

            MINI = 128

            def feat_part(dst, off, src, c0, c1):
                psF = pbig.tile([128, c1 - c0], F32, tag="pbig", name="psF")
                nc.tensor.matmul(psF, lhsT=wT_s, rhs=src, start=True, stop=True)
                nc.scalar.activation(dst[:, off + c0:off + c1], psF,
                                     AF.Prelu, alpha=NEG_SLOPE)

            def feat_chunk(q):  # q = 1..3 -> cols [512q, 512q+512) of x/mem
                for dst, off, m in ((fiT, 0, 0), (fmT, HALF, 1)):
                    if q == 1:
                        src = xmA1_s[:, CH * m:CH * (m + 1)]
                    else:
                        # xmB packed as [x c2 | mem c2 | x c3 | mem c3]
                        src = xmB_s[:, 1024 * (q - 2) + CH * m:
                                    1024 * (q - 2) + CH * (m + 1)]
                    psF = pbig.tile([128, CH], F32, tag="pbig", name="psF")
                    nc.tensor.matmul(psF, lhsT=wT_s, rhs=src, start=True, stop=True)
                    nc.scalar.activation(dst[:, off + CH * q:off + CH * (q + 1)],
                                         psF, AF.Prelu, alpha=NEG_SLOPE)

            # chunk 0 in two parts: a 128-col mini first so group 0's QK
            # unblocks early, then the 384-col remainder
            feat_part(fiT, 0, xmA0a_s[:, H:H + MINI], 0, MINI)
            feat_part(fmT, HALF, xmA0b_s[:, 0:MINI], 0, MINI)
            feat_part(fiT, 0, xmA0a_s[:, H + MINI:H + CH], MINI, CH)
            feat_part(fmT, HALF, xmA0b_s[:, MINI:CH], MINI, CH)

            # group 0's QK only needs chunk 0, so emit it ahead of chunk 1
            # in the PE stream
            g0t0, g0K = GROUPS[0]
            psA_g0 = pbig.tile([RT, g0K, WIN], F32, tag="pbig", name="psA")
            for k, t in enumerate(range(g0t0, g0t0 + g0K)):
                nc.tensor.matmul(psA_g0[:, k, :], lhsT=fiT[:, RT * t:RT * (t + 1)],
                                 rhs=fmT[:, RT * t:RT * t + WIN],
                                 start=True, stop=True)
            feat_chunk(1)
            nc.scalar.dma_start(out=xmB_s, in_=xmB.ap())

            # ---- banded attention in groups ----
            for g, (t0, K) in enumerate(GROUPS):
                if g in (1, 2):
                    feat_chunk(g + 1)

                tiles = list(range(t0, t0 + K))

                if g == 0:
                    psA = psA_g0
                else:
                    psA = pbig.tile([RT, K, WIN], F32, tag="pbig", name="psA")
                    for k, t in enumerate(tiles):
                        nc.tensor.matmul(psA[:, k, :],
                                         lhsT=fiT[:, RT * t:RT * (t + 1)],
                                         rhs=fmT[:, RT * t:RT * t + WIN],
                                         start=True, stop=True)

                # fused (psA + mask) + row-max, per tile
                if USE_TTR:
                    for k, t in enumerate(tiles):
                        mik = 0 if t == 0 else (2 if t == T - 1 else 1)
                        nc.vector.tensor_tensor_reduce(
                            out=simS[:, t, :], in0=psA[:, k, :],
                            in1=mi_s[:, mik, :], scale=1.0, scalar=-1e30,
                            op0=ALU.add, op1=ALU.max,
                            accum_out=rowmax[:, t:t + 1])
                else:
                    runs = []  # (k_start, count, mi)
                    for k, t in enumerate(tiles):
                        mik = 0 if t == 0 else (2 if t == T - 1 else 1)
                        if runs and runs[-1][2] == mik:
                            runs[-1][1] += 1
                        else:
                            runs.append([k, 1, mik])
                    for k0, cnt, mik in runs:
                        if cnt == 1:
                            nc.vector.tensor_tensor(
                                simS[:, t0 + k0:t0 + k0 + 1, :],
                                psA[:, k0:k0 + 1, :],
                                mi_s[:, mik:mik + 1, :], ALU.add)
                        else:
                            nc.vector.tensor_tensor(
                                simS[:, t0 + k0:t0 + k0 + cnt, :],
                                psA[:, k0:k0 + cnt, :],
                                bcast_mid(mi_s[:, mik:mik + 1, :], cnt), ALU.add)
                    nc.vector.reduce_max(
                        rowmax[:, t0:t0 + K], simS[:, t0:t0 + K, :],
                        axis=AX.X)
                # simB = simS - rowmax (batched, bf16 out)
                nc.gpsimd.tensor_tensor(
                    simB[:, t0:t0 + K, :], simS[:, t0:t0 + K, :],
                    bcast(rowmax[:, t0:t0 + K], WIN), ALU.subtract)
                nc.scalar.activation(EB[:, t0:t0 + K, :], simB[:, t0:t0 + K, :],
                                     AF.Exp)

                psT = ptc.tile([128, K, RT], BF16, tag="ptc")
                for k, t in enumerate(tiles):
                    nc.tensor.transpose(psT[:, k, :], EB[:, t, :], ident)
                attnT = work.tile([128, K, RT], BF16)
                if USE_VCOPY:
                    nc.vector.tensor_scalar_add(attnT[:, 0:K, :], psT[:, 0:K, :], 0.0)
                else:
                    nc.scalar.copy(attnT[:, 0:K, :], psT[:, 0:K, :])

                psC = ptc.tile([RT, K, L + 1], F32, tag="ptc")
                for k, t in enumerate(tiles):
                    nc.tensor.matmul(psC[:, k, :], lhsT=attnT[:, k, :],
                                     rhs=memNB_s[:, t, :], start=True, stop=True)

                # epilogue: out = psC * (1/denom) + 0.5*x, batched
                nc.vector.reciprocal(rinv[:, t0:t0 + K], psC[:, 0:K, L])
                nc.vector.tensor_tensor(
                    tmpn[:, t0:t0 + K, :], psC[:, 0:K, 0:L],
                    bcast(rinv[:, t0:t0 + K], L), ALU.mult)
                nc.gpsimd.tensor_tensor(
                    outn[:, t0:t0 + K, :], tmpn[:, t0:t0 + K, :],
                    xh_s[:, t0:t0 + K, :], ALU.add)

                nc.sync.dma_start(
                    out=out.ap().rearrange("p (t d) -> p t d", d=L)[:, t0:t0 + K, :],
                    in_=outn[:, t0:t0 + K, :])

    nc.compile()
    return nc


def _host_prep(input, state_memory, conv_w, conv_b, bn_gamma, bn_beta, bn_mean, bn_var):
    from ml_dtypes import bfloat16

    s = (bn_gamma / np.sqrt(bn_var + BN_EPS)).astype(np.float32)
    bias_h = ((conv_b - bn_mean) * s + bn_beta).astype(np.float32)
    # W' = W * s (per output channel), bias as 57th contraction row
    wT = (conv_w[:, 0, :].T * s[None, :]).astype(np.float32)           # [L, H]
    wT_ext = np.concatenate(
        [wT, bias_h[None, :], np.zeros((LEP - LE, H), np.float32)], axis=0)

    # Per-tile additive mask [RT, WIN] (0 in band, -1e10 outside)
    def tile_mask(t):
        m = np.full((RT, WIN), -1e10, dtype=np.float32)
        for r in range(RT):
            i = RT * t + r
            if i >= N:
                continue
            lo = max(i - HALF, 0) - (RT * t - HALF)
            hi = min(i + HALF, N - 1) - (RT * t - HALF)
            m[r, lo:hi + 1] = 0.0
        return m

    maskD = np.stack([tile_mask(0), tile_mask(1), tile_mask(T - 1)], axis=1)
    maskD = maskD.reshape(RT, -1)                                      # [RT, 3*WIN]
    mi_host = np.concatenate([maskD, np.eye(RT, dtype=np.float32)], axis=1)

    in_maps = []
    for b in range(B):
        x = np.ascontiguousarray(input[b, :, 0, :]).astype(np.float32)
        mem = np.ascontiguousarray(state_memory[b, :, 0, :]).astype(np.float32)
        ones = np.ones((1, N), dtype=np.float32)
        zpad = np.zeros((LEP - LE, N), dtype=np.float32)
        xT = np.concatenate([x.T, ones, zpad], axis=0)                 # [LEP, N]
        memT = np.concatenate([mem.T, ones, zpad], axis=0)             # [LEP, N]
        # window-aligned mem blocks: block t = rows [RT*t-8, RT*t+120)
        mnb = np.zeros((T, 128, L + 1), dtype=np.float32)
        half_mem = (1.0 - RATE) * mem
        for t in range(T):
            lo = RT * t - HALF
            a, bnd = max(0, lo), min(N, lo + 128)
            if a < bnd:
                mnb[t, a - lo:bnd - lo, 0:L] = half_mem[a:bnd]
                mnb[t, a - lo:bnd - lo, L] = 1.0
        xh = np.zeros((T, RT, L), dtype=np.float32)
        xh.reshape(-1, L)[:N] = RATE * x
        in_maps.append({
            "xmA0a": np.ascontiguousarray(np.concatenate(
                [wT_ext, xT[:, 0:512]], axis=1)).astype(np.float16),
            "xmA0b": np.ascontiguousarray(memT[:, 0:512]).astype(np.float16),
            "xmA1": np.ascontiguousarray(np.concatenate(
                [xT[:, 512:1024], memT[:, 512:1024]], axis=1)).astype(np.float16),
            "xmB": np.ascontiguousarray(np.concatenate(
                [xT[:, 1024:1536], memT[:, 1024:1536],
                 xT[:, 1536:2048], memT[:, 1536:2048]], axis=1)).astype(np.float16),
            "mi": np.ascontiguousarray(mi_host).astype(bfloat16),
            "memNB": np.ascontiguousarray(
                mnb.transpose(1, 0, 2).reshape(128, -1)).astype(bfloat16),
            "xhp": np.ascontiguousarray(
                xh.transpose(1, 0, 2).reshape(RT, -1)).astype(np.float16),
        })
    return in_maps


def run(inputs, trace=False):
    from concourse.bass_utils import run_bass_kernel_spmd
    if "nc" not in _cache:
        _cache["nc"] = _build_program()
    nc = _cache["nc"]
    in_maps = _host_prep(**inputs)
    res = run_bass_kernel_spmd(nc, in_maps, core_ids=list(range(B)), trace=trace)
    out = np.empty((B, N, C, L), dtype=np.float32)
    for b in range(B):
        o = res.results[b]["out"].astype(np.float32).reshape(RT, T, L).transpose(1, 0, 2)
        out[b] = o.reshape(NPAD, L)[:N].reshape(N, C, L)
    return out, res


def kernel(**inputs):
    out, _ = run(inputs, trace=False)
    return out
